# revision 12
# baseline (speedup 1.0000x reference)
"""BoxFuse (sparse_attention) Trainium2 Bass kernel, v2.

Data-parallel over batch: 32 batches -> 8 NeuronCores x 4 batches.
All projections run as fp8(e4m3) DoubleRow matmuls (0.5 cyc/row), with
weights pre-scaled by a power-of-2 (drain un-scales). Activations are
LN-normalized on ScalarE (f32 stats on VectorE), PE-transposed in bf16,
and cast to fp8 during the batched PSUM->SBUF copy. The attention scale
1/sqrt(d) is folded into the softmax mask multiplier, att/att@v run in
bf16. Epilogue: ScalarE drains att@v PSUM with the 1/rowsum scale,
GpSimd adds the residual in SBUF, so VectorE only does LN stats +
transpose copies.
"""

import os
import numpy as np

# The Bass kernel executes via the axon/neuron jax platform; a stray
# JAX_PLATFORMS=cpu (set for running the jax reference) would hide the
# NeuronCores from the runtime.
if os.environ.get("JAX_PLATFORMS", "").strip() == "cpu":
    os.environ.pop("JAX_PLATFORMS")

B, NTOK, L, LOW, HIGH = 32, 576, 100, 1024, 1536
NCORES = 8
BPC = B // NCORES            # batches per core
LN_EPS = 1e-5
MASK_NEG = -30.0
ATT_SCALE = 1.0 / 32.0       # 1/sqrt(LOW)
DT = LOW // 128              # 8 d-tiles of output features
NKB_Q = LOW // 256           # 4 double-row k-tiles for vit
NKB_KV = HIGH // 256         # 6 double-row k-tiles for box
LB = BPC * L                 # 400: batch-concat box token dim

_CACHE = {}


def _build(reps=1):
    import concourse.bacc as bacc
    import concourse.tile as tile
    import concourse.mybir as mybir

    F32 = mybir.dt.float32
    BF16 = mybir.dt.bfloat16
    FP8 = mybir.dt.float8e4
    AF = mybir.ActivationFunctionType
    ALU = mybir.AluOpType
    DRM = mybir.MatmulPerfMode.DoubleRow

    nc = bacc.Bacc("TRN2", target_bir_lowering=False, debug=False)

    vit_d = nc.dram_tensor("vit", [BPC, NTOK, LOW], F32, kind="ExternalInput").ap()
    box_d = nc.dram_tensor("box", [BPC, L, HIGH], F32, kind="ExternalInput").ap()
    qw_d = nc.dram_tensor("qw", [128, NKB_Q, 2, LOW], FP8, kind="ExternalInput").ap()
    kw_d = nc.dram_tensor("kw", [128, NKB_KV, 2, LOW], FP8, kind="ExternalInput").ap()
    vw_d = nc.dram_tensor("vw", [128, NKB_KV, 2, LOW], FP8, kind="ExternalInput").ap()
    qb_d = nc.dram_tensor("qb", [128, DT], F32, kind="ExternalInput").ap()
    kb_d = nc.dram_tensor("kb", [128, DT], F32, kind="ExternalInput").ap()
    vb_d = nc.dram_tensor("vb", [128, DT], F32, kind="ExternalInput").ap()
    msc_d = nc.dram_tensor("msc", [128, BPC], F32, kind="ExternalInput").ap()
    mbs_d = nc.dram_tensor("mbs", [128, BPC], F32, kind="ExternalInput").ap()
    id16_d = nc.dram_tensor("id16", [128, 128], BF16, kind="ExternalInput").ap()
    ones_d = nc.dram_tensor("ones", [128, 1], BF16, kind="ExternalInput").ap()
    wsc_d = nc.dram_tensor("wsc", [128, 3], F32, kind="ExternalInput").ap()
    out_d = nc.dram_tensor("out", [BPC, NTOK, LOW], F32, kind="ExternalOutput").ap()

    NT = [(t * 128, min(128, NTOK - t * 128)) for t in range(5)]

    with tile.TileContext(nc) as tc:
      for _rep in range(reps):
        with (
            tc.tile_pool(name="consts", bufs=1) as consts,
            tc.tile_pool(name="wpool", bufs=1) as wpool,
            tc.tile_pool(name="persist", bufs=1) as persist,
            tc.tile_pool(name="small", bufs=int(os.environ.get("BF_SM", "12"))) as small,
            tc.tile_pool(name="bxp", bufs=2) as bxp,
            tc.tile_pool(name="xhp", bufs=int(os.environ.get("BF_XH", "4"))) as xhp,
            tc.tile_pool(name="vTp", bufs=1) as vTp,
            tc.tile_pool(name="vitp", bufs=int(os.environ.get("BF_VIT", "2"))) as vitp,
            tc.tile_pool(name="xtp", bufs=int(os.environ.get("BF_XT", "3"))) as xtp,
            tc.tile_pool(name="qtp", bufs=int(os.environ.get("BF_QT", "3"))) as qtp,
            tc.tile_pool(name="attp", bufs=2) as attp,
            tc.tile_pool(name="outp", bufs=int(os.environ.get("BF_OUT", "4"))) as outp,
            tc.tile_pool(name="pp_t", bufs=int(os.environ.get("BF_PPT", "2")), space="PSUM") as pp_t,
            tc.tile_pool(name="pp_j", bufs=int(os.environ.get("BF_PJ", "2")), space="PSUM") as pp_j,
            tc.tile_pool(name="pp_a", bufs=int(os.environ.get("BF_PA", "2")), space="PSUM") as pp_a,
            tc.tile_pool(name="pp_v", bufs=int(os.environ.get("BF_PV", "2")), space="PSUM") as pp_v,
        ):
            id16 = consts.tile([128, 128], BF16, tag="id16")
            nc.sync.dma_start(id16[:], id16_d)
            ones = consts.tile([128, 1], BF16, tag="ones")
            nc.sync.dma_start(ones[:], ones_d)
            msc = consts.tile([128, BPC], F32, tag="msc")
            nc.sync.dma_start(msc[:], msc_d)
            mbs = consts.tile([128, BPC], F32, tag="mbs")
            nc.sync.dma_start(mbs[:], mbs_d)
            qb = consts.tile([128, DT], F32, tag="qb")
            nc.sync.dma_start(qb[:], qb_d)
            kb = consts.tile([128, DT], F32, tag="kb")
            nc.sync.dma_start(kb[:], kb_d)
            vb = consts.tile([128, DT], F32, tag="vb")
            nc.sync.dma_start(vb[:], vb_d)
            # per-partition broadcast of the three 1/WS drain scales
            wsc = consts.tile([128, 3], F32, tag="wsc")
            nc.sync.dma_start(wsc[:], wsc_d)

            eps_t = consts.tile([128, 1], F32, tag="eps")
            nc.vector.memset(eps_t[:], LN_EPS)

            def layernorm_stats(x_ap, rows, width):
                """x_ap: [rows, width] f32 in SBUF -> (r, nmr) [rows, 1]."""
                chunks = width // 512
                st6 = small.tile([128, chunks, 6], F32, tag="st6")
                for c in range(chunks):
                    nc.vector.bn_stats(
                        st6[:rows, c, :], x_ap[:rows, c * 512:(c + 1) * 512]
                    )
                st2 = small.tile([128, 2], F32, tag="st2")
                nc.vector.bn_aggr(st2[:rows, :], st6[:rows, :, :])
                sd = small.tile([128, 1], F32, tag="sd")
                nc.scalar.activation(sd[:rows, :], st2[:rows, 1:2], AF.Sqrt,
                                     bias=eps_t[:rows, :], scale=1.0)
                r = small.tile([128, 1], F32, tag="r")
                nc.vector.reciprocal(r[:rows, :], sd[:rows, :])
                nmr = small.tile([128, 1], F32, tag="nmr")
                nc.vector.scalar_tensor_tensor(
                    nmr[:rows, :], st2[:rows, 0:1], -1.0, r[:rows, :],
                    op0=ALU.mult, op1=ALU.mult,
                )
                return r, nmr

            # ---------------- Phase A: box -> boxT8, kT, vnat ----------------
            boxT8 = persist.tile([128, NKB_KV, 2, LB], FP8, tag="boxT8")

            def box_batch(b):
                bx = bxp.tile([128, HIGH], F32, tag="bx")
                nc.sync.dma_start(bx[:L, :], box_d[b])
                r, nmr = layernorm_stats(bx, L, HIGH)
                xh = xhp.tile([128, HIGH], BF16, tag="xhb")
                nc.scalar.activation(xh[:L, :], bx[:L, :], AF.Identity,
                                     bias=nmr[:L, :], scale=r[:L, :])
                for half in range(2):
                    pt = pp_t.tile([128, 8, 128], BF16, tag="pt")
                    for hh in range(6):
                        h = half * 6 + hh
                        nc.tensor.transpose(
                            pt[:, hh, :L], xh[:L, h * 128:(h + 1) * 128],
                            id16[:L, :L],
                        )
                    nc.vector.tensor_copy(
                        boxT8[:, half * 3:(half + 1) * 3, :, b * L:(b + 1) * L],
                        pt[:, :6, :L].rearrange("p (kb i) n -> p kb i n", i=2),
                    )

            for b in range(BPC):
                box_batch(b)

            kw = wpool.tile([128, NKB_KV, 2, LOW], FP8, tag="kw")
            nc.sync.dma_start(kw[:], kw_d)
            vw = wpool.tile([128, NKB_KV, 2, LOW], FP8, tag="vw")
            nc.sync.dma_start(vw[:], vw_d)
            qw = wpool.tile([128, NKB_Q, 2, LOW], FP8, tag="qw")
            nc.sync.dma_start(qw[:], qw_d)

            # kT[d-part, d-tile, l-concat] = (kw^T @ box_lnT) * 1/WSK + kb
            kT = persist.tile([128, DT, LB], BF16, tag="kT")
            for d in range(DT):
                ps = pp_j.tile([128, 512], F32, tag="pj")
                for kbi in range(NKB_KV):
                    nc.tensor.matmul(
                        ps[:, :LB], kw[:, kbi, :, d * 128:(d + 1) * 128],
                        boxT8[:, kbi, :, :], start=(kbi == 0),
                        stop=(kbi == NKB_KV - 1), perf_mode=DRM,
                    )
                nc.scalar.activation(kT[:, d, :], ps[:, :LB], AF.Identity,
                                     bias=kb[:, d:d + 1], scale=wsc[:, 1:2])

            # v^T, then transpose to v-natural
            vT = vTp.tile([128, DT, LB], BF16, tag="vT")
            for d in range(DT):
                ps = pp_j.tile([128, 512], F32, tag="pj")
                for kbi in range(NKB_KV):
                    nc.tensor.matmul(
                        ps[:, :LB], vw[:, kbi, :, d * 128:(d + 1) * 128],
                        boxT8[:, kbi, :, :], start=(kbi == 0),
                        stop=(kbi == NKB_KV - 1), perf_mode=DRM,
                    )
                nc.scalar.activation(vT[:, d, :], ps[:, :LB], AF.Identity,
                                     bias=vb[:, d:d + 1], scale=wsc[:, 2:3])
            vnat = persist.tile([128, BPC, LOW], BF16, tag="vnat")
            for dp in range(DT // 2):
                ptv = pp_t.tile([128, 8, 128], BF16, tag="pt")
                for dd in range(2):
                    for b in range(BPC):
                        nc.tensor.transpose(
                            ptv[:L, dd * BPC + b, :],
                            vT[:, dp * 2 + dd, b * L:(b + 1) * L], id16[:, :],
                        )
                nc.vector.tensor_copy(
                    vnat[:L, :, dp * 256:(dp + 1) * 256].rearrange(
                        "p b (dd n) -> p dd b n", n=128),
                    ptv[:L].rearrange("p (dd b) n -> p dd b n", b=BPC),
                )

            # ---------------- Phase B: per batch ----------------
            def vit_batch(b):
                vitn = vitp.tile([128, 5, LOW], F32, tag="vit")
                for t, (st, w) in enumerate(NT):
                    nc.sync.dma_start(vitn[:w, t, :], vit_d[b, st:st + w, :])

                xT8 = xtp.tile([128, NKB_Q, 2, NTOK], FP8, tag="xT8")
                for t, (st, w) in enumerate(NT):
                    r, nmr = layernorm_stats(vitn[:, t, :], w, LOW)
                    xh = xhp.tile([128, LOW], BF16, tag="xhv")
                    nc.scalar.activation(xh[:w, :], vitn[:w, t, :],
                                         AF.Identity, bias=nmr[:w, :],
                                         scale=r[:w, :])
                    ptx = pp_t.tile([128, 8, 128], BF16, tag="pt")
                    for h in range(DT):
                        nc.tensor.transpose(
                            ptx[:, h, :w], xh[:w, h * 128:(h + 1) * 128],
                            id16[:w, :w],
                        )
                    nc.vector.tensor_copy(
                        xT8[:, :, :, st:st + w],
                        ptx[:, :, :w].rearrange("p (kb i) n -> p kb i n", i=2),
                    )

                # q^T = (qw^T @ x_hatT) * 1/WSQ + qb
                qT = qtp.tile([128, DT, NTOK], BF16, tag="qT")
                for d in range(DT):
                    for cs, cw in ((0, 512), (512, 64)):
                        ps = pp_j.tile([128, 512], F32, tag="pj")
                        for kbi in range(NKB_Q):
                            nc.tensor.matmul(
                                ps[:, :cw], qw[:, kbi, :, d * 128:(d + 1) * 128],
                                xT8[:, kbi, :, cs:cs + cw], start=(kbi == 0),
                                stop=(kbi == NKB_Q - 1), perf_mode=DRM,
                            )
                        nc.scalar.activation(qT[:, d, cs:cs + cw],
                                             ps[:, :cw], AF.Identity,
                                             bias=qb[:, d:d + 1],
                                             scale=wsc[:, 0:1])

                # attT[l, n] = exp((k . q) * msc + mbs)   (msc folds 1/sqrt(d))
                attT = attp.tile([128, NTOK], BF16, tag="attT")
                for cs in (0, 288):
                    psa = pp_a.tile([128, 288], F32, tag="att")
                    for d in range(DT):
                        nc.tensor.matmul(
                            psa[:L, :], kT[:, d, b * L:(b + 1) * L],
                            qT[:, d, cs:cs + 288],
                            start=(d == 0), stop=(d == DT - 1),
                        )
                    nc.scalar.activation(attT[:L, cs:cs + 288], psa[:L, :],
                                         AF.Exp, bias=mbs[:L, b:b + 1],
                                         scale=msc[:L, b:b + 1])

                # rowsums + recips first: inv ready before the av loop
                inv = small.tile([128, 5], F32, tag="inv")
                for s, (st, w) in enumerate(NT):
                    pss = pp_a.tile([128, 288], F32, tag="att")
                    nc.tensor.matmul(pss[:w, :1], attT[:L, st:st + w],
                                     ones[:L, :], start=True, stop=True)
                    nc.vector.reciprocal(inv[:w, s:s + 1], pss[:w, :1])
                # att@v + epilogue per n-slice
                for s, (st, w) in enumerate(NT):
                    outst = outp.tile([128, LOW], F32, tag="outst")
                    for c in range(2):
                        cs = c * 512
                        psv = pp_v.tile([128, 512], F32, tag="av")
                        nc.tensor.matmul(
                            psv[:w, :], attT[:L, st:st + w],
                            vnat[:L, b, cs:cs + 512], start=True, stop=True,
                        )
                        nc.scalar.activation(outst[:w, cs:cs + 512], psv[:w, :],
                                             AF.Identity,
                                             scale=inv[:w, s:s + 1])
                    # residual add on GpSimd (SBUF-only)
                    nc.gpsimd.tensor_add(outst[:w, :], outst[:w, :],
                                         vitn[:w, s, :])
                    nc.gpsimd.dma_start(out_d[b, st:st + w, :], outst[:w, :])

            for b in range(BPC):
                vit_batch(b)

    nc.compile()
    return nc


def _pow2_scale(w, target=224.0):
    m = float(np.abs(w).max())
    if m <= 0:
        return 1.0
    return float(2.0 ** np.floor(np.log2(target / m)))


def kernel(**inputs):
    import ml_dtypes
    from concourse.bass_utils import run_bass_kernel_spmd

    f32 = np.float32
    E4M3 = ml_dtypes.float8_e4m3
    BF16NP = ml_dtypes.bfloat16

    vit = np.ascontiguousarray(inputs["vit_feat"], dtype=f32)
    box = np.ascontiguousarray(inputs["box_feat"], dtype=f32)
    lengths = np.asarray(inputs["lengths"])

    def eff(ln_w, ln_b, w, bias):
        w = np.asarray(w, f32)
        weff = np.asarray(ln_w, f32)[:, None] * w
        beff = np.asarray(ln_b, f32) @ w + np.asarray(bias, f32)
        return weff, beff

    qw, qbv = eff(inputs["q_ln_w"], inputs["q_ln_b"], inputs["q_w"], inputs["q_b"])
    kw, kbv = eff(inputs["k_ln_w"], inputs["k_ln_b"], inputs["k_w"], inputs["k_b"])
    vw, vbv = eff(inputs["v_ln_w"], inputs["v_ln_b"], inputs["v_w"], inputs["v_b"])

    wsq, wsk, wsv = (_pow2_scale(w) for w in (qw, kw, vw))

    def pack8(w, ws, nkb):
        # [K, M] -> [p, kb, i, m] with k = (2*kb + i)*128 + p
        w8 = (w * f32(ws)).astype(E4M3)
        return np.ascontiguousarray(
            w8.reshape(nkb, 2, 128, LOW).transpose(2, 0, 1, 3))

    qw8 = pack8(qw, wsq, NKB_Q)
    kw8 = pack8(kw, wsk, NKB_KV)
    vw8 = pack8(vw, wsv, NKB_KV)

    def packb(bv):
        return np.ascontiguousarray(bv.reshape(DT, 128).T.astype(f32))

    valid = (np.arange(L)[None, :] < lengths[:, None].astype(np.int64))  # [B, L]
    msc_all = np.where(valid, f32(ATT_SCALE), f32(0.0))
    mbs_all = np.where(valid, f32(0.0), f32(MASK_NEG))
    # pad L=100 -> 128 partitions (rows >= L are never read, pad as masked)
    msc_pad = np.zeros((B, 128), f32)
    mbs_pad = np.full((B, 128), f32(MASK_NEG))
    msc_pad[:, :L] = msc_all
    mbs_pad[:, :L] = mbs_all

    id16 = np.eye(128, dtype=BF16NP)
    ones = np.ones((128, 1), dtype=BF16NP)
    wsc = np.tile(np.array([[1.0 / wsq, 1.0 / wsk, 1.0 / wsv]], f32), (128, 1))
    wsc = np.ascontiguousarray(wsc)

    if "nc" not in _CACHE:
        _CACHE["nc"] = _build()
    nc = _CACHE["nc"]

    in_maps = []
    for c in range(NCORES):
        sl = slice(c * BPC, (c + 1) * BPC)
        in_maps.append({
            "vit": vit[sl], "box": box[sl],
            "qw": qw8, "kw": kw8, "vw": vw8,
            "qb": packb(qbv), "kb": packb(kbv), "vb": packb(vbv),
            "msc": np.ascontiguousarray(msc_pad[sl].T),
            "mbs": np.ascontiguousarray(mbs_pad[sl].T),
            "id16": id16, "ones": ones, "wsc": wsc,
        })

    _CACHE["in_maps"] = in_maps
    res = run_bass_kernel_spmd(nc, in_maps, core_ids=list(range(NCORES)))
    out = np.concatenate([res.results[c]["out"] for c in range(NCORES)], axis=0)
    return np.ascontiguousarray(out.astype(np.float32))


if __name__ == "__main__":
    rng = np.random.default_rng(0)
    ins = {
        "vit_feat": rng.standard_normal((B, NTOK, LOW)).astype(np.float32),
        "box_feat": rng.standard_normal((B, L, HIGH)).astype(np.float32),
        "lengths": rng.integers(0, L, (B,)).astype(np.int64),
        "q_ln_w": np.ones(LOW, np.float32), "q_ln_b": np.zeros(LOW, np.float32),
        "q_w": (rng.standard_normal((LOW, LOW)) * 0.02).astype(np.float32),
        "q_b": np.zeros(LOW, np.float32),
        "k_ln_w": np.ones(HIGH, np.float32), "k_ln_b": np.zeros(HIGH, np.float32),
        "k_w": (rng.standard_normal((HIGH, LOW)) * 0.02).astype(np.float32),
        "k_b": np.zeros(LOW, np.float32),
        "v_ln_w": np.ones(HIGH, np.float32), "v_ln_b": np.zeros(HIGH, np.float32),
        "v_w": (rng.standard_normal((HIGH, LOW)) * 0.02).astype(np.float32),
        "v_b": np.zeros(LOW, np.float32),
    }
    out = kernel(**ins)
    print("out", out.shape, out.dtype, np.abs(out).mean())
